# revision 28
# baseline (speedup 1.0000x reference)
# Trainium2 Bass kernel for nn_Attention_88313117540497.
#
# Reference computation (per batch b of 128):
#   v = x_b @ Wv                      (196, 384) @ (384, 512)
#   conv: each of the 512 channels' 14x14 image convolved with a 27x27
#         kernel qk at padding 13 -> same 14x14 output
#   y = conv_out @ Wo + bo            (196, 512) @ (512, 384)
#
# Math restructuring:
#  1. The 27x27 kernel at padding 13 covers every input pixel for every
#     output pixel, so the conv is a dense 196x196 linear map M over
#     positions, shared by all batches/channels: conv == matmul.
#  2. Folding W = Wv @ Wo (384x384) removes INNER=512:
#     y_b = (M @ x_b) @ W + bo.
#  3. All-transposed, M-first dataflow minimizes PE streaming cycles:
#       Z.T[d,p] = sum_u X[u,d] MT[u,p]   (lhsT = X chunk, rhs = MT)
#       Y.T[e,p] = sum_d W[d,e] Z.T[d,p]  (lhsT = W tile,  rhs = Z.T)
#     2940 PE cycles/batch vs 3840 for the W-first token-major order.
#  4. bf16 everywhere (4.4e-3 max-normalized error vs the 2e-2 budget):
#     1 cycle/row at any N (f32r is 4 cyc/row under N=256), single fast
#     LDWEIGHTS per matmul (fp32-family needs a LOW/HIGH pair - the
#     baseline's PE throttle), half the HBM bytes both directions.
#
# Schedule (from trace analysis of the first bf16 cut):
#  - x is host-packed feature-transposed and token-PADDED to 256 so every
#    load is a single 128-partition DMA with one contiguous slab per
#    partition (the unpadded 68-row token tail otherwise needs its own
#    slow scattered transfers; zero rows contribute nothing to the
#    contraction).
#  - MT rides the fast sync HWDGE ring FIRST (it gates every stage-1
#    matmul; on SWDGE it landed at 12.2us and stalled the PE 3.8us),
#    W on the scalar HWDGE ring, bias on SWDGE.
#  - ZT/YT are software-pipelined one group apart in emission order so
#    the PE never waits for the scalar-engine PSUM evictions of the
#    group it just computed (PE streams are executed in order).
#  - PSUM evictions: stage-1 on scalar (ACT copy+cast), stage-2 on
#    vector (tensor_scalar bias-add+cast), ~587/619ns per 392-elem op,
#    each engine ~14.5us < PE ~21us.
#  - y stores: 2-group grains on sync/gpsimd mid-kernel; the last two
#    groups go per-group on the two HWDGE rings only (a final SWDGE
#    store costs ~2.5us of Q7 descriptor tail).
#
# Sharding: data-parallel over batch, 16 batches per core, weights
# replicated. No collectives.

import numpy as np
import ml_dtypes

import concourse.bass as bass
from concourse import bacc
import concourse.mybir as mybir
import concourse.tile as tile
from concourse.bass_utils import run_bass_kernel_spmd

N_CORES = 8
B = 128                 # total batch
BPC = B // N_CORES      # batches per core
DIM = 384
NPOS = 196              # 14*14 positions
IMG = 14
KS = 27                 # conv kernel size
TPAD = 256              # tokens padded to 2 full partition chunks
U1 = NPOS - 128         # valid rows in token chunk 1 (68)

F32 = mybir.dt.float32
BF16 = mybir.dt.bfloat16
NP_BF16 = ml_dtypes.bfloat16

NG = BPC // 2           # 2-batch compute groups
GW = 2 * NPOS           # output cols per group: 392
BW = 2 * DIM            # packed x cols per batch: 768
# (start batch, count): all on the sync HWDGE ring (SWDGE bulk loads
# measured 5+us slower and stall the PE mid-stream). Progressive sizes:
# tiny leading groups minimize first-data latency, and the serialized
# ~0.65us triggers naturally stagger the bulk so the early small
# transfers' completion receipts aren't stuck behind megabytes of
# in-flight packets.
XGROUPS = [(0, 1), (1, 1), (2, 1), (3, 1), (4, 2), (6, 2), (8, 4), (12, 4)]
NXG = len(XGROUPS)


def build_program():
    nc = bacc.Bacc("TRN2", debug=False)

    # x packed: xp[p, b*768 + c*384 + d] = x[b, 128c+p, d] (0 for pad rows)
    xp_d = nc.dram_tensor("xp", [128, BPC * BW], BF16, kind="ExternalInput")
    # MT packed: cols 0:196 = MT[0:128,:]; cols 196:392 = MT[128:196,:]
    # on rows 0:68, zeros on rows 68:128 (pad tokens)
    mt_d = nc.dram_tensor("mt", [128, GW], BF16, kind="ExternalInput")
    # W folded, tiled: block k*3+j = W[128k:128k+128, 128j:128j+128]
    w_d = nc.dram_tensor("w", [128, 9 * 128], BF16, kind="ExternalInput")
    bias_d = nc.dram_tensor("bias", [128, 3], F32, kind="ExternalInput")
    # y transposed: [e-chunk, e%128, batch-token stream]
    y_d = nc.dram_tensor("y", [3, 128, BPC * NPOS], BF16, kind="ExternalOutput")

    with tile.TileContext(nc) as tc:
        with (
            tc.tile_pool(name="const", bufs=1) as const,
            tc.tile_pool(name="work", bufs=2) as work,
            tc.tile_pool(name="psum", bufs=2, space="PSUM") as psum,
        ):
            # ---- PE warm-up feeder: memset first thing on gpsimd (its
            # preamble ends ~6.2us) so the wide warm matmuls can start
            # ~6.6us with no DMA in their dependency chain ----
            warm_sb = const.tile([128, 256], BF16)
            nc.gpsimd.memset(warm_sb[:, :], 1.0)

            # ---- constants: mt gates stage 1 -> scalar ring, first in
            # time; w second on scalar; bias on SWDGE. Keeping the early
            # fabric window clear of bulk x traffic is critical - if all
            # x groups issue at once, mt's completion receipt is delayed
            # ~4us behind 2.4MB of in-flight packets (measured). ----
            mt_sb = const.tile([128, GW], BF16)
            nc.scalar.dma_start(mt_sb[:, :], mt_d[:, :])
            w_sb = const.tile([128, 9 * 128], BF16)
            nc.scalar.dma_start(w_sb[:, :], w_d[:, :])
            bias_sb = const.tile([128, 3], F32)
            nc.gpsimd.dma_start(bias_sb[:, :], bias_d[:, :])

            # ---- x loads: one contiguous 128-partition DMA per group;
            # first three (small) on sync, back half on SWDGE ----
            # groups 4-5 (batches 4-7) ride the scalar ring behind mt/w:
            # splitting the load chain across both HWDGE rings shortens
            # each ring's serialized completion cadence so every group
            # lands just ahead of its (clock-warm) consumption time
            xp_t = {}
            for gi, (s, nb) in enumerate(XGROUPS):
                eng = nc.scalar if gi in (4, 5) else nc.sync
                t = work.tile([128, nb * BW], BF16, tag="xp", bufs=NXG,
                              name=f"xp{gi}")
                eng.dma_start(t[:, 0:nb * BW],
                              xp_d[:, s * BW:(s + nb) * BW])
                for b in range(s, s + nb):
                    xp_t[b] = (t, (b - s) * BW)

            # ---- PE warm-up: WIDE (N=512) matmuls on the memset tile,
            # bridging from the preamble to the first data matmul with no
            # idle gap. The HAM clock-gate needs ~3.4us of sustained PE
            # *streaming* activity - N=1 const-AP matmuls never qualify
            # (measured: 30 of them left the clock at 1.2GHz well into
            # the data stream). Nine 512-column matmuls keep the array
            # streaming ~100% from ~6.7us so the 2.4GHz unthrottle lands
            # at or before the first data matmul (~10.4us). ----
            for wi in range(18):
                warm = psum.tile([128, 256], F32, tag="z0", name=f"warm{wi}")
                nc.tensor.matmul(
                    warm[0:1, :], lhsT=warm_sb[:, 0:1], rhs=warm_sb[:, :],
                    start=True, stop=True,
                )

            # ---- main loop: ZT(g) emitted one group ahead of YT(g) ----
            ZBUFS = [2, 2, 1]
            zsb_g = {}

            def emit_zt(g):
                ba, bb = 2 * g, 2 * g + 1
                zps = [psum.tile([128, GW], F32, tag=f"z{k}",
                                 bufs=ZBUFS[k], name=f"zp{k}_{g}")
                       for k in range(3)]
                # batch-outer emission: all six of batch a's matmuls run
                # before batch b's first, so b's load may land ~0.5us
                # later without stalling the PE
                for half, b in ((0, ba), (1, bb)):
                    t, off = xp_t[b]
                    c0 = half * NPOS
                    for k in range(3):
                        nc.tensor.matmul(
                            zps[k][:, c0:c0 + NPOS],
                            lhsT=t[:, off + k * 128:off + (k + 1) * 128],
                            rhs=mt_sb[:, 0:NPOS],
                            start=True, stop=False,
                        )
                        nc.tensor.matmul(
                            zps[k][:, c0:c0 + NPOS],
                            lhsT=t[:, off + DIM + k * 128:
                                   off + DIM + (k + 1) * 128],
                            rhs=mt_sb[:, NPOS:GW],
                            start=False, stop=True,
                        )
                zsb = []
                for k in range(3):
                    z = work.tile([128, GW], BF16, tag=f"zsb{k}", bufs=2,
                                  name=f"zsb{k}_{g}")
                    # stage-1 evictions on scalar (ACT copy + cast)
                    nc.scalar.copy(z[:, :], zps[k][:, :])
                    zsb.append(z)
                zsb_g[g] = zsb

            ysb = {}

            def emit_yt(g):
                zsb = zsb_g.pop(g)
                pair, half = g // 2, g % 2
                last2 = g >= NG - 2
                for j in range(3):
                    yp = psum.tile([128, GW], F32, tag=f"y{j}", bufs=1,
                                   name=f"yp{j}_{g}")
                    for k in range(3):
                        nc.tensor.matmul(
                            yp[:, :],
                            lhsT=w_sb[:, (k * 3 + j) * 128:
                                      (k * 3 + j + 1) * 128],
                            rhs=zsb[k][:, :],
                            start=(k == 0), stop=(k == 2),
                        )
                    if last2:
                        yt = work.tile([128, GW], BF16, tag=f"ysb{j}", bufs=2,
                                       name=f"ysb{j}_{g}")
                        dst = yt[:, 0:GW]
                    else:
                        if half == 0:
                            ysb[j] = work.tile([128, 2 * GW], BF16,
                                               tag=f"ysb{j}", bufs=2,
                                               name=f"ysb{j}_{pair}")
                        yt = ysb[j]
                        dst = yt[:, half * GW:(half + 1) * GW]
                    # stage-2 evictions on vector (bias add + cast); for
                    # the final two groups spread j=1 to scalar so the
                    # last eviction chain (which gates the final store
                    # receipts) is two ops deep instead of three
                    final = g == NG - 1
                    if final and j == 2:
                        # very last tile: evict in halves on two engines
                        # in parallel, store halves on the two HWDGE
                        # rings - the tail is evict/2 + trigger + receipt
                        nc.vector.tensor_scalar_add(
                            dst[:, 0:NPOS], yp[:, 0:NPOS],
                            bias_sb[:, j:j + 1])
                        nc.scalar.activation(
                            dst[:, NPOS:GW], yp[:, NPOS:GW],
                            mybir.ActivationFunctionType.Identity,
                            bias=bias_sb[:, j:j + 1],
                        )
                        nc.sync.dma_start(
                            y_d[j, :, g * GW:g * GW + NPOS],
                            yt[:, 0:NPOS])
                        nc.scalar.dma_start(
                            y_d[j, :, g * GW + NPOS:(g + 1) * GW],
                            yt[:, NPOS:GW])
                        continue
                    if last2 and j == 1:
                        nc.scalar.activation(
                            dst, yp[:, :],
                            mybir.ActivationFunctionType.Identity,
                            bias=bias_sb[:, j:j + 1],
                        )
                    else:
                        nc.vector.tensor_scalar_add(dst, yp[:, :],
                                                    bias_sb[:, j:j + 1])
                    if last2:
                        # small final transfers, HWDGE rings only (a
                        # SWDGE store here leaves a ~2.4us Q7 tail)
                        eng = (nc.sync, nc.scalar, nc.sync)[j]
                        eng.dma_start(
                            y_d[j, :, g * GW:(g + 1) * GW], yt[:, 0:GW])
                    elif half == 1:
                        eng = (nc.sync, nc.sync, nc.gpsimd)[j]
                        eng.dma_start(
                            y_d[j, :, pair * 2 * GW:(pair + 1) * 2 * GW],
                            yt[:, 0:2 * GW])

            emit_zt(0)
            for g in range(NG):
                if g + 1 < NG:
                    emit_zt(g + 1)
                emit_yt(g)

    nc.compile()
    return nc


_PROGRAM = None


def _get_program():
    global _PROGRAM
    if _PROGRAM is None:
        _PROGRAM = build_program()
    return _PROGRAM


def _host_prep(x, Wv, qk, Wo, bo):
    x = np.asarray(x, dtype=np.float32)
    xc = x.reshape(N_CORES, BPC, NPOS, DIM)
    # xp[core, p, b, c, d] = x[core, b, 128c+p, d], pad rows zero
    xpad = np.zeros((N_CORES, BPC, 2, 128, DIM), np.float32)
    xpad[:, :, 0, :, :] = xc[:, :, 0:128, :]
    xpad[:, :, 1, 0:U1, :] = xc[:, :, 128:NPOS, :]
    xp = np.ascontiguousarray(
        xpad.transpose(0, 3, 1, 2, 4).reshape(N_CORES, 128, BPC * BW)
    ).astype(NP_BF16)
    # W = Wv @ Wo folded once, tiled [128, 9*128] with block k*3+j
    W = (np.asarray(Wv, np.float32) @ np.asarray(Wo, np.float32))
    wt = np.ascontiguousarray(
        W.reshape(3, 128, 3, 128).transpose(1, 0, 2, 3).reshape(128, 9 * 128)
    ).astype(NP_BF16)
    # MT[(u,v),(p,q)] = qk[13+u-p, 13+v-q]
    qk2 = np.asarray(qk, np.float32).reshape(KS, KS)
    idx = (KS // 2) + np.arange(IMG)[:, None] - np.arange(IMG)[None, :]
    MT = qk2[idx[:, None, :, None], idx[None, :, None, :]].reshape(NPOS, NPOS)
    mt = np.zeros((128, GW), np.float32)
    mt[:, 0:NPOS] = MT[0:128, :]
    mt[0:U1, NPOS:GW] = MT[128:NPOS, :]
    mt = mt.astype(NP_BF16)
    bias = np.zeros((128, 3), np.float32)
    bias[:, 0] = np.asarray(bo, np.float32)[0:128]
    bias[:, 1] = np.asarray(bo, np.float32)[128:256]
    bias[:, 2] = np.asarray(bo, np.float32)[256:384]
    return xp, wt, mt, bias


def _unpack_core(y2):
    # y2: [3, 128, BPC*NPOS] bf16 -> (BPC, NPOS, DIM) fp32
    return np.ascontiguousarray(
        np.asarray(y2).reshape(3, 128, BPC, NPOS).transpose(2, 3, 0, 1)
        .reshape(BPC, NPOS, DIM)
    ).astype(np.float32)


def _run(x, Wv, qk, Wo, bo, **spmd_kwargs):
    xp, wt, mt, bias = _host_prep(x, Wv, qk, Wo, bo)
    nc = _get_program()
    in_maps = [
        {"xp": xp[c], "w": wt, "mt": mt, "bias": bias}
        for c in range(N_CORES)
    ]
    res = run_bass_kernel_spmd(nc, in_maps, list(range(N_CORES)), **spmd_kwargs)
    y = np.concatenate(
        [_unpack_core(res.results[c]["y"]) for c in range(N_CORES)], axis=0)
    return y, res


def kernel(x, Wv, qk, Wo, bo):
    y, _ = _run(x, Wv, qk, Wo, bo)
    return y
